# revision 1
# baseline (speedup 1.0000x reference)
"""DisentangledAttention on 8 Trainium2 cores (Bass/Tile).

Problem (hardcoded shapes): x[2,2048,1024], 16 heads x 64 dims, torch-Linear
projections, scores = q@k^T/8 + alpha_h*(pe@pe^T) + beta_h, key-side mask,
softmax, out = (P@v) @ Wo^T + bo.

Sharding: core i = (batch b = i//4, head-group g = i%4, heads 4g..4g+3).
Each core computes its 4 heads' attention and a partial out-projection
[2048,1024]; host sums the 4 partials per batch (tensor-parallel unshard).

Math simplifications (exact):
- beta_h is constant along the softmax axis -> cancels. Dropped.
- bk shifts scores by q.bk, constant along key axis -> cancels. Dropped.
- bv contributes sum_k P[q,k] * (bv @ Wo_slice^T) = bv @ Wo^T per row since
  softmax rows sum to 1 -> exact host-side additive correction with bo.
- bq enters scores via bq.k -> per-partition scalar add on q^T (DVE).
- 1/sqrt(64) folded into Wq on host; alpha_h applied on device (ACT scale).
- pos term fused into the QK matmul: q' = [q/8 ; alpha_h*pe], k' = [k ; pe]
  stacked along the contraction dim (64+64=128) -> pos attention is free.
- scores are built TRANSPOSED [key, query]: the key-side mask becomes a
  per-partition ACT bias on the exp, and P~^T feeds the PV matmul directly
  (no transpose). Softmax denominators come from a ones-row appended to V
  (M=65 PV matmul); normalization is a reciprocal + GPSIMD partition
  broadcast + multiply, entirely off the PE's critical path.
- no max-subtraction in softmax: scores ~ N(0,1) here, exp is f32-safe, and
  softmax is shift-invariant so this matches the reference to rounding.

Matmuls run in float32r (TF32-like, 11 mantissa bits, 4x faster than fp32
on the PE). PSUM accumulation is fp32. The out-projection is interleaved
into the (ACT-exp-paced) attention stream through a unified 4-slot PSUM
pool, making it nearly free on the PE timeline.
"""

import numpy as np

import concourse.bacc as bacc
import concourse.bass as bass
import concourse.mybir as mybir
import concourse.tile as tile
from concourse.bass import ds, ts
from concourse.bass_utils import run_bass_kernel_spmd

B = 2
S = 2048
D = 1024
H = 16
DH = 64
NCORES = 8
GROUPS = 4          # head-groups
HPC = H // GROUPS   # heads per core = 4
O = HPC * DH        # out dims per core = 256
KD = D // 128       # contraction tiles over d_model = 8
ST = S // 128       # seq tiles of 128 = 16
SC = S // 512       # seq chunks of 512 = 4

F32 = mybir.dt.float32
F32R = mybir.dt.float32r

_CACHE = {}


def _build(reps: int = 1):
    nc = bacc.Bacc("TRN2", target_bir_lowering=False, debug=False, num_devices=NCORES)

    # x / weights arrive pre-laid-out by the host in exactly the SBUF tile
    # shape ([partition, k-tile, free]), so each loads with ONE fully
    # contiguous DMA
    xT = nc.dram_tensor("xT", [128, KD, S], F32, kind="ExternalInput").ap()
    wqT = nc.dram_tensor("wqT", [128, KD, O], F32, kind="ExternalInput").ap()
    wkT = nc.dram_tensor("wkT", [128, KD, O], F32, kind="ExternalInput").ap()
    wvT = nc.dram_tensor("wvT", [128, KD, O], F32, kind="ExternalInput").ap()
    woT = nc.dram_tensor("woT", [128, 2, D], F32, kind="ExternalInput").ap()
    peT = nc.dram_tensor("peT", [DH, S], F32, kind="ExternalInput").ap()
    # smalls: [:, 0:2] = bq/8 by head-pair, [:, 2:18] = mask bias by key
    # tile, [0:64, 18:22] = per-head alpha replicated down 64 partitions
    smalls_d = nc.dram_tensor("smalls", [128, 2 + ST + HPC], F32, kind="ExternalInput").ap()
    out_d = nc.dram_tensor("out", [S, D], F32, kind="ExternalOutput").ap()

    with tile.TileContext(nc) as tc:
      for _rep in range(reps):
        with tc.tile_pool(name="const", bufs=1) as const:
            woT_sb = const.tile([128, 2, D], F32R)
            smalls = const.tile([128, 2 + ST + HPC], F32)
            bqsb = smalls[:, 0:2]
            maskb = smalls[:, 2 : 2 + ST]
            alphas = smalls[0:64, 2 + ST : 2 + ST + HPC]
            peT_sb = const.tile([DH, S], F32)

            # q'/k' per head ([128, S]: content half + pos half stacked along
            # the contraction dim), V' per head ([128 keys, 64+1] per key tile)
            with tc.tile_pool(name="qkv", bufs=1) as qkv:
                qp = [qkv.tile([128, S], F32R, name=f"qp{h}") for h in range(HPC)]
                kp = [qkv.tile([128, S], F32R, name=f"kp{h}") for h in range(HPC)]
                vp = qkv.tile([128, ST, HPC, DH + 1], F32R)
                nc.vector.memset(vp[:, :, :, DH : DH + 1].bitcast(F32), 1.0)

                # ---- projections ----
                with tc.tile_pool(name="proj", bufs=1) as proj:
                    xT_sb = proj.tile([128, KD, S], F32R)
                    wq_sb = proj.tile([128, KD, O], F32R)
                    wk_sb = proj.tile([128, KD, O], F32R)
                    wv_sb = proj.tile([128, KD, O], F32R)
                    # DMA order = need order: x/weights gate the first
                    # matmuls; pos-embed and Wo are consumed much later
                    nc.sync.dma_start(out=xT_sb, in_=xT.bitcast(F32R))
                    for w_sb, w_d in ((wq_sb, wqT), (wk_sb, wkT), (wv_sb, wvT)):
                        nc.sync.dma_start(out=w_sb, in_=w_d.bitcast(F32R))
                    nc.sync.dma_start(out=smalls, in_=smalls_d)
                    nc.sync.dma_start(out=peT_sb, in_=peT)
                    # pos halves: even head of a pair keeps content in rows
                    # 0:64 / pos in 64:128, odd head the reverse (both sides
                    # of the QK matmul use the same order, so dots match).
                    # q side is scaled by the head's alpha (data-driven)
                    for h in range(HPC):
                        crow = (h % 2) * 64          # content rows base
                        prow = 64 - crow             # pos rows base
                        nc.scalar.mul(
                            qp[h][prow : prow + 64, :],
                            peT_sb,
                            alphas[:, h : h + 1],
                        )
                        nc.scalar.copy(
                            out=kp[h][prow : prow + 64, :],
                            in_=peT_sb,
                        )
                    nc.sync.dma_start(out=woT_sb, in_=woT.bitcast(F32R))

                    with tc.tile_pool(name="pps", bufs=4, space="PSUM") as pps:
                        # q^T and k^T by head-pair: psum [128 (2 heads), 512]
                        for hp in range(2):
                            for c in range(SC):
                                q_ps = pps.tile([128, 512], F32, tag="qk_ps")
                                for kd in range(KD):
                                    nc.tensor.matmul(
                                        out=q_ps,
                                        lhsT=wq_sb[:, kd, ts(hp, 128)],
                                        rhs=xT_sb[:, kd, ds(c * 512, 512)],
                                        start=(kd == 0),
                                        stop=(kd == KD - 1),
                                    )
                                for par in range(2):  # even/odd head of pair
                                    h = 2 * hp + par
                                    crow = (h % 2) * 64
                                    nc.vector.tensor_scalar_add(
                                        qp[h][crow : crow + 64, ds(c * 512, 512)],
                                        q_ps[crow : crow + 64, :],
                                        bqsb[crow : crow + 64, hp : hp + 1],
                                    )
                                k_ps = pps.tile([128, 512], F32, tag="qk_ps")
                                for kd in range(KD):
                                    nc.tensor.matmul(
                                        out=k_ps,
                                        lhsT=wk_sb[:, kd, ts(hp, 128)],
                                        rhs=xT_sb[:, kd, ds(c * 512, 512)],
                                        start=(kd == 0),
                                        stop=(kd == KD - 1),
                                    )
                                for par in range(2):
                                    h = 2 * hp + par
                                    crow = (h % 2) * 64
                                    nc.vector.tensor_copy(
                                        out=kp[h][crow : crow + 64, ds(c * 512, 512)],
                                        in_=k_ps[crow : crow + 64, :],
                                    )
                        # v: [seq, o] directly
                        for st in range(ST):
                            v_ps = pps.tile([128, O], F32, tag="v_ps")
                            for kd in range(KD):
                                nc.tensor.matmul(
                                    out=v_ps,
                                    lhsT=xT_sb[:, kd, ts(st, 128)],
                                    rhs=wv_sb[:, kd, :],
                                    start=(kd == 0),
                                    stop=(kd == KD - 1),
                                )
                            nc.vector.tensor_copy(
                                out=vp[:, st, :, 0:DH],
                                in_=v_ps.rearrange("p (h d) -> p h d", h=HPC),
                            )

                # ---- attention (scores transposed [key, query]) ----
                # sq handled in chunks of 1024 (CW) so exp runs as [128, 1024]
                # ACT instructions, halving the per-instruction access bubble
                CW = 1024
                NCH = S // CW
                attnT = [qkv.tile([128, S], F32R, name=f"attnT{kt}") for kt in range(2)]
                # single unified PSUM pool: scores, z-accumulators, and
                # out-proj tiles are all 2 banks, so one 4-slot pool (8 banks)
                # lets outproj cycle through the spare slot without starving
                # the score ping-pong that paces ACT
                with (
                    tc.tile_pool(name="att", bufs=3) as att,
                    tc.tile_pool(name="nrm", bufs=2) as nrm,
                    tc.tile_pool(name="ups", bufs=4, space="PSUM") as ups,
                ):
                    def emit_outproj(st_range):
                        # partial out-projection (host sums over head-groups).
                        # Emitted per sq half as soon as all heads' attnT
                        # columns are done: this PE work runs inside the
                        # ACT(exp)-paced attention stream, so it's ~free.
                        # o_ps shares the score pool slots (free between
                        # chunks) to stay within the 8 PSUM banks. Two
                        # s-tiles share one staging tile and one DMA.
                        sts = list(st_range)
                        for st0 in sts[::2]:
                            o_sb2 = att.tile([128, 2, D], F32, tag="osb2")
                            for j in range(2):
                                st = st0 + j
                                o_ps = ups.tile([128, D], F32, tag="u", name="o_ps")
                                for nk in range(2):
                                    for kt in range(2):
                                        nc.tensor.matmul(
                                            out=o_ps[:, ds(nk * 512, 512)],
                                            lhsT=attnT[kt][:, ts(st, 128)],
                                            rhs=woT_sb[:, kt, ds(nk * 512, 512)],
                                            start=(kt == 0),
                                            stop=(kt == 1),
                                        )
                                nc.vector.tensor_copy(out=o_sb2[:, j, :], in_=o_ps)
                            nc.sync.dma_start(
                                out=out_d[ds(st0 * 128, 256), :].rearrange(
                                    "(two p) d -> p two d", p=128
                                ),
                                in_=o_sb2,
                            )

                    for c in range(NCH):
                        for h in range(HPC):
                            if c > 0 and h > 0:
                                # previous chunk's outproj, spread in small
                                # waves one head into this chunk: the
                                # dependency (previous chunk's last
                                # normalize) is long done, and small waves
                                # limit score-slot contention
                                w0, w1 = [(0, 0), (0, 2), (2, 6), (6, 8)][h]
                                emit_outproj(range((c - 1) * 8 + w0, (c - 1) * 8 + w1))
                            z_ps = ups.tile([DH + 1, CW], F32, tag="u", name="z_ps")
                            for t in range(ST):
                                s_ps = ups.tile([128, CW], F32, tag="u", name="s_ps")
                                for half in range(CW // 512):
                                    nc.tensor.matmul(
                                        out=s_ps[:, ds(half * 512, 512)],
                                        lhsT=kp[h][:, ts(t, 128)],
                                        rhs=qp[h][:, ds(c * CW + half * 512, 512)],
                                        start=True,
                                        stop=True,
                                    )
                                p_sb = att.tile([128, CW], F32R, tag="p")
                                nc.scalar.activation(
                                    out=p_sb,
                                    in_=s_ps,
                                    func=mybir.ActivationFunctionType.Exp,
                                    bias=maskb[:, t : t + 1],
                                    scale=1.0,
                                )
                                for half in range(CW // 512):
                                    nc.tensor.matmul(
                                        out=z_ps[:, ds(half * 512, 512)],
                                        lhsT=vp[:, t, h, :],
                                        rhs=p_sb[:, ds(half * 512, 512)],
                                        start=(t == 0),
                                        stop=(t == ST - 1),
                                    )
                            recip = nrm.tile([1, CW], F32, tag="recip")
                            nc.vector.reciprocal(recip, z_ps[DH : DH + 1, :])
                            # broadcast 1/denom to 64 partitions on GPSIMD
                            # (keeps the whole normalize chain off the PE)
                            bc_sb = nrm.tile([64, CW], F32, tag="bc_sb")
                            nc.gpsimd.partition_broadcast(bc_sb, recip)
                            row = (h % 2) * 64
                            nc.vector.tensor_mul(
                                out=attnT[h // 2][row : row + 64, ds(c * CW, CW)],
                                in0=z_ps[0:DH, :],
                                in1=bc_sb,
                            )
                    emit_outproj(range((NCH - 1) * 8, NCH * 8))

    nc.compile()
    return nc


def kernel(
    x, mask, Wq, bq, Wk, bk, Wv, bv, Wo, bo, pos_embed, alpha, beta, **_unused
):
    x = np.asarray(x, dtype=np.float32)
    mask = np.asarray(mask)
    Wq = np.asarray(Wq, dtype=np.float32)
    Wk = np.asarray(Wk, dtype=np.float32)
    Wv = np.asarray(Wv, dtype=np.float32)
    Wo = np.asarray(Wo, dtype=np.float32)
    bq = np.asarray(bq, dtype=np.float32)
    bv = np.asarray(bv, dtype=np.float32)
    bo = np.asarray(bo, dtype=np.float32)
    pe = np.asarray(pos_embed, dtype=np.float32)
    alpha = np.asarray(alpha, dtype=np.float32).reshape(H)

    if "nc" not in _CACHE:
        _CACHE["nc"] = _build()
    nc = _CACHE["nc"]

    scale = np.float32(1.0 / np.sqrt(DH))
    peT_np = np.ascontiguousarray(pe.T)
    maskbias = np.where(mask == 0, np.float32(-1e9), np.float32(0.0)).astype(np.float32)

    in_maps = []
    for core in range(NCORES):
        b, g = divmod(core, GROUPS)
        osl = slice(g * O, (g + 1) * O)
        heads = list(range(g * HPC, (g + 1) * HPC))
        smalls = np.zeros((128, 2 + ST + HPC), np.float32)
        smalls[:, 0:2] = (bq[osl] * scale).reshape(2, 128).T
        smalls[:, 2 : 2 + ST] = maskbias[b].reshape(ST, 128).T
        smalls[0:64, 2 + ST :] = alpha[heads][None, :]
        def sb_layout(mat_T, kt):
            # [rows, cols] -> [128, kt, cols]: row r = k*128 + p -> [p][k]
            r, cols = mat_T.shape
            return np.ascontiguousarray(
                mat_T.reshape(kt, 128, cols).transpose(1, 0, 2)
            )

        in_maps.append(
            {
                "xT": sb_layout(x[b].T, KD),
                "wqT": sb_layout((Wq[osl] * scale).T, KD),
                "wkT": sb_layout(Wk[osl].T, KD),
                "wvT": sb_layout(Wv[osl].T, KD),
                "woT": sb_layout(Wo[:, osl].T, 2),
                "peT": peT_np,
                "smalls": smalls,
                "out": np.zeros((S, D), np.float32),
            }
        )

    _CACHE["in_maps"] = in_maps
    res = run_bass_kernel_spmd(nc, in_maps, core_ids=list(range(NCORES)))

    correction = Wo @ bv + bo  # exact bv/bo contribution (see module docstring)
    out = np.empty((B, S, D), np.float32)
    for b in range(B):
        acc = np.zeros((S, D), np.float64)
        for g in range(GROUPS):
            acc += res.results[b * GROUPS + g]["out"]
        out[b] = (acc + correction).astype(np.float32)
    return out



# revision 4
# speedup vs baseline: 625.4450x; 625.4450x over previous
"""DisentangledAttention on 8 Trainium2 cores (Bass/Tile).

Problem (hardcoded shapes): x[2,2048,1024], 16 heads x 64 dims, torch-Linear
projections, scores = q@k^T/8 + alpha_h*(pe@pe^T) + beta_h, key-side mask,
softmax, out = (P@v) @ Wo^T + bo.

Sharding: core i = (batch b = i//4, head-group g = i%4, heads 4g..4g+3).
Each core computes its 4 heads' attention and a partial out-projection
[2048,1024]; host sums the 4 partials per batch (tensor-parallel unshard).

Math simplifications (exact):
- beta_h is constant along the softmax axis -> cancels. Dropped.
- bk shifts scores by q.bk, constant along key axis -> cancels. Dropped.
- bv contributes sum_k P[q,k] * (bv @ Wo_slice^T) = bv @ Wo^T per row since
  softmax rows sum to 1 -> exact host-side additive correction with bo.
- bq enters scores via bq.k -> per-partition scalar add on q^T (DVE).
- 1/sqrt(64) folded into Wq on host; alpha_h applied on device (ACT scale).
- pos term fused into the QK matmul: q' = [q/8 ; alpha_h*pe], k' = [k ; pe]
  stacked along the contraction dim (64+64=128) -> pos attention is free.
- scores are built TRANSPOSED [key, query]: the key-side mask becomes a
  per-partition ACT bias on the exp, and P~^T feeds the PV matmul directly
  (no transpose). Softmax denominators come from a ones-row appended to V
  (M=65 PV matmul); normalization is a reciprocal + GPSIMD partition
  broadcast + multiply, entirely off the PE's critical path.
- no max-subtraction in softmax: scores ~ N(0,1) here, exp is f32-safe, and
  softmax is shift-invariant so this matches the reference to rounding.

Matmuls run in float32r (TF32-like, 11 mantissa bits, 4x faster than fp32
on the PE). PSUM accumulation is fp32. The out-projection is interleaved
into the (ACT-exp-paced) attention stream through a unified 4-slot PSUM
pool, making it nearly free on the PE timeline.
"""

import numpy as np

import concourse.bacc as bacc
import concourse.bass as bass
import concourse.mybir as mybir
import concourse.tile as tile
from concourse.bass import ds, ts
from concourse.bass_utils import run_bass_kernel_spmd

B = 2
S = 2048
D = 1024
H = 16
DH = 64
NCORES = 8
GROUPS = 4          # head-groups
HPC = H // GROUPS   # heads per core = 4
O = HPC * DH        # out dims per core = 256
KD = D // 128       # contraction tiles over d_model = 8
ST = S // 128       # seq tiles of 128 = 16
SC = S // 512       # seq chunks of 512 = 4

F32 = mybir.dt.float32
F32R = mybir.dt.float32r

_CACHE = {}


class _null_ctx:
    def __enter__(self):
        return None

    def __exit__(self, *a):
        return False


def _build(reps: int = 1, loop_n: int | None = None):
    """Build the kernel program.

    reps: unrolled copies of the body (bench uses >1 to amortize).
    loop_n: if set, wrap the reps in a For_i hardware loop of this count
    (bench-only — lets timing loops run long enough to dominate the
    ~30-90ms tunnel dispatch jitter without exploding program size).
    """
    nc = bacc.Bacc("TRN2", target_bir_lowering=False, debug=False, num_devices=NCORES)

    # x / weights arrive pre-laid-out by the host in exactly the SBUF tile
    # shape ([partition, k-tile, free]), so each loads with ONE fully
    # contiguous DMA
    xT = nc.dram_tensor("xT", [128, KD, S], F32, kind="ExternalInput").ap()
    wqT = nc.dram_tensor("wqT", [128, KD, O], F32, kind="ExternalInput").ap()
    wkT = nc.dram_tensor("wkT", [128, KD, O], F32, kind="ExternalInput").ap()
    wvT = nc.dram_tensor("wvT", [128, KD, O], F32, kind="ExternalInput").ap()
    woT = nc.dram_tensor("woT", [128, 2, D], F32, kind="ExternalInput").ap()
    peT = nc.dram_tensor("peT", [DH, S], F32, kind="ExternalInput").ap()
    # smalls: [:, 0:2] = bq/8 by head-pair, [:, 2:18] = mask bias by key
    # tile, [0:64, 18:22] = per-head alpha replicated down 64 partitions
    smalls_d = nc.dram_tensor("smalls", [128, 2 + ST + HPC], F32, kind="ExternalInput").ap()
    out_d = nc.dram_tensor("out", [S, D], F32, kind="ExternalOutput").ap()

    with tile.TileContext(nc) as tc:
     with tc.For_i(0, loop_n, 1) if loop_n else _null_ctx() as _i:
      for _rep in range(reps):
        with tc.tile_pool(name="const", bufs=1) as const:
            woT_sb = const.tile([128, 2, D], F32R)
            smalls = const.tile([128, 2 + ST + HPC], F32)
            bqsb = smalls[:, 0:2]
            maskb = smalls[:, 2 : 2 + ST]
            alphas = smalls[0:64, 2 + ST : 2 + ST + HPC]
            peT_sb = const.tile([DH, S], F32)

            # q'/k' per head ([128, S]: content half + pos half stacked along
            # the contraction dim), V' per head ([128 keys, 64+1] per key tile)
            with tc.tile_pool(name="qkv", bufs=1) as qkv:
                qp = [qkv.tile([128, S], F32R, name=f"qp{h}") for h in range(HPC)]
                kp = [qkv.tile([128, S], F32R, name=f"kp{h}") for h in range(HPC)]
                vp = qkv.tile([128, ST, HPC, DH + 1], F32R)
                nc.vector.memset(vp[:, :, :, DH : DH + 1].bitcast(F32), 1.0)

                # ---- projections ----
                with tc.tile_pool(name="proj", bufs=1) as proj:
                    xT_sb = proj.tile([128, KD, S], F32R)
                    wq_sb = proj.tile([128, KD, O], F32R)
                    wk_sb = proj.tile([128, KD, O], F32R)
                    wv_sb = proj.tile([128, KD, O], F32R)
                    # DMA order = need order: x/weights gate the first
                    # matmuls; pos-embed and Wo are consumed much later
                    nc.sync.dma_start(out=xT_sb, in_=xT.bitcast(F32R))
                    for w_sb, w_d in ((wq_sb, wqT), (wk_sb, wkT), (wv_sb, wvT)):
                        nc.sync.dma_start(out=w_sb, in_=w_d.bitcast(F32R))
                    nc.sync.dma_start(out=smalls, in_=smalls_d)
                    nc.sync.dma_start(out=peT_sb, in_=peT)
                    # pos halves: even head of a pair keeps content in rows
                    # 0:64 / pos in 64:128, odd head the reverse (both sides
                    # of the QK matmul use the same order, so dots match).
                    # q side is scaled by the head's alpha (data-driven)
                    for h in range(HPC):
                        crow = (h % 2) * 64          # content rows base
                        prow = 64 - crow             # pos rows base
                        nc.scalar.mul(
                            qp[h][prow : prow + 64, :],
                            peT_sb,
                            alphas[:, h : h + 1],
                        )
                        nc.scalar.copy(
                            out=kp[h][prow : prow + 64, :],
                            in_=peT_sb,
                        )
                    nc.sync.dma_start(out=woT_sb, in_=woT.bitcast(F32R))

                    with tc.tile_pool(name="pps", bufs=4, space="PSUM") as pps:
                        # q^T and k^T by head-pair: psum [128 (2 heads), 512]
                        for hp in range(2):
                            for c in range(SC):
                                q_ps = pps.tile([128, 512], F32, tag="qk_ps")
                                for kd in range(KD):
                                    nc.tensor.matmul(
                                        out=q_ps,
                                        lhsT=wq_sb[:, kd, ts(hp, 128)],
                                        rhs=xT_sb[:, kd, ds(c * 512, 512)],
                                        start=(kd == 0),
                                        stop=(kd == KD - 1),
                                    )
                                for par in range(2):  # even/odd head of pair
                                    h = 2 * hp + par
                                    crow = (h % 2) * 64
                                    nc.vector.tensor_scalar_add(
                                        qp[h][crow : crow + 64, ds(c * 512, 512)],
                                        q_ps[crow : crow + 64, :],
                                        bqsb[crow : crow + 64, hp : hp + 1],
                                    )
                                k_ps = pps.tile([128, 512], F32, tag="qk_ps")
                                for kd in range(KD):
                                    nc.tensor.matmul(
                                        out=k_ps,
                                        lhsT=wk_sb[:, kd, ts(hp, 128)],
                                        rhs=xT_sb[:, kd, ds(c * 512, 512)],
                                        start=(kd == 0),
                                        stop=(kd == KD - 1),
                                    )
                                for par in range(2):
                                    h = 2 * hp + par
                                    crow = (h % 2) * 64
                                    nc.vector.tensor_copy(
                                        out=kp[h][crow : crow + 64, ds(c * 512, 512)],
                                        in_=k_ps[crow : crow + 64, :],
                                    )
                        # v: [seq, o] directly
                        for st in range(ST):
                            v_ps = pps.tile([128, O], F32, tag="v_ps")
                            for kd in range(KD):
                                nc.tensor.matmul(
                                    out=v_ps,
                                    lhsT=xT_sb[:, kd, ts(st, 128)],
                                    rhs=wv_sb[:, kd, :],
                                    start=(kd == 0),
                                    stop=(kd == KD - 1),
                                )
                            nc.vector.tensor_copy(
                                out=vp[:, st, :, 0:DH],
                                in_=v_ps.rearrange("p (h d) -> p h d", h=HPC),
                            )

                # ---- attention (scores transposed [key, query]) ----
                # sq handled in chunks of 1024 (CW) so exp runs as [128, 1024]
                # ACT instructions, halving the per-instruction access bubble
                CW = 1024
                NCH = S // CW
                attnT = [qkv.tile([128, S], F32R, name=f"attnT{kt}") for kt in range(2)]
                # single unified PSUM pool: scores, z-accumulators, and
                # out-proj tiles are all 2 banks, so one 4-slot pool (8 banks)
                # lets outproj cycle through the spare slot without starving
                # the score ping-pong that paces ACT
                with (
                    tc.tile_pool(name="att", bufs=3) as att,
                    tc.tile_pool(name="nrm", bufs=2) as nrm,
                    tc.tile_pool(name="ups", bufs=4, space="PSUM") as ups,
                ):
                    def emit_outproj(st_range):
                        # partial out-projection (host sums over head-groups).
                        # Emitted per sq half as soon as all heads' attnT
                        # columns are done: this PE work runs inside the
                        # ACT(exp)-paced attention stream, so it's ~free.
                        # o_ps shares the score pool slots (free between
                        # chunks) to stay within the 8 PSUM banks. Two
                        # s-tiles share one staging tile and one DMA.
                        sts = list(st_range)
                        for st0 in sts[::2]:
                            o_sb2 = att.tile([128, 2, D], F32, tag="osb2")
                            for j in range(2):
                                st = st0 + j
                                o_ps = ups.tile([128, D], F32, tag="u", name="o_ps")
                                for nk in range(2):
                                    for kt in range(2):
                                        nc.tensor.matmul(
                                            out=o_ps[:, ds(nk * 512, 512)],
                                            lhsT=attnT[kt][:, ts(st, 128)],
                                            rhs=woT_sb[:, kt, ds(nk * 512, 512)],
                                            start=(kt == 0),
                                            stop=(kt == 1),
                                        )
                                nc.vector.tensor_copy(out=o_sb2[:, j, :], in_=o_ps)
                            nc.sync.dma_start(
                                out=out_d[ds(st0 * 128, 256), :].rearrange(
                                    "(two p) d -> p two d", p=128
                                ),
                                in_=o_sb2,
                            )

                    for c in range(NCH):
                        for h in range(HPC):
                            if c > 0 and h > 0:
                                # previous chunk's outproj, spread in small
                                # waves one head into this chunk: the
                                # dependency (previous chunk's last
                                # normalize) is long done, and small waves
                                # limit score-slot contention
                                w0, w1 = [(0, 0), (0, 2), (2, 6), (6, 8)][h]
                                emit_outproj(range((c - 1) * 8 + w0, (c - 1) * 8 + w1))
                            z_ps = ups.tile([DH + 1, CW], F32, tag="u", name="z_ps")
                            for t in range(ST):
                                s_ps = ups.tile([128, CW], F32, tag="u", name="s_ps")
                                for half in range(CW // 512):
                                    nc.tensor.matmul(
                                        out=s_ps[:, ds(half * 512, 512)],
                                        lhsT=kp[h][:, ts(t, 128)],
                                        rhs=qp[h][:, ds(c * CW + half * 512, 512)],
                                        start=True,
                                        stop=True,
                                    )
                                p_sb = att.tile([128, CW], F32R, tag="p")
                                nc.scalar.activation(
                                    out=p_sb,
                                    in_=s_ps,
                                    func=mybir.ActivationFunctionType.Exp,
                                    bias=maskb[:, t : t + 1],
                                    scale=1.0,
                                )
                                for half in range(CW // 512):
                                    nc.tensor.matmul(
                                        out=z_ps[:, ds(half * 512, 512)],
                                        lhsT=vp[:, t, h, :],
                                        rhs=p_sb[:, ds(half * 512, 512)],
                                        start=(t == 0),
                                        stop=(t == ST - 1),
                                    )
                            recip = nrm.tile([1, CW], F32, tag="recip")
                            nc.vector.reciprocal(recip, z_ps[DH : DH + 1, :])
                            # broadcast 1/denom to 64 partitions on GPSIMD
                            # (keeps the whole normalize chain off the PE)
                            bc_sb = nrm.tile([64, CW], F32, tag="bc_sb")
                            nc.gpsimd.partition_broadcast(bc_sb, recip)
                            row = (h % 2) * 64
                            nc.vector.tensor_mul(
                                out=attnT[h // 2][row : row + 64, ds(c * CW, CW)],
                                in0=z_ps[0:DH, :],
                                in1=bc_sb,
                            )
                    emit_outproj(range((NCH - 1) * 8, NCH * 8))

    nc.compile()
    return nc


def kernel(
    x, mask, Wq, bq, Wk, bk, Wv, bv, Wo, bo, pos_embed, alpha, beta, **_unused
):
    x = np.asarray(x, dtype=np.float32)
    mask = np.asarray(mask)
    Wq = np.asarray(Wq, dtype=np.float32)
    Wk = np.asarray(Wk, dtype=np.float32)
    Wv = np.asarray(Wv, dtype=np.float32)
    Wo = np.asarray(Wo, dtype=np.float32)
    bq = np.asarray(bq, dtype=np.float32)
    bv = np.asarray(bv, dtype=np.float32)
    bo = np.asarray(bo, dtype=np.float32)
    pe = np.asarray(pos_embed, dtype=np.float32)
    alpha = np.asarray(alpha, dtype=np.float32).reshape(H)

    if "nc" not in _CACHE:
        _CACHE["nc"] = _build()
    nc = _CACHE["nc"]

    scale = np.float32(1.0 / np.sqrt(DH))
    peT_np = np.ascontiguousarray(pe.T)
    maskbias = np.where(mask == 0, np.float32(-1e9), np.float32(0.0)).astype(np.float32)

    in_maps = []
    for core in range(NCORES):
        b, g = divmod(core, GROUPS)
        osl = slice(g * O, (g + 1) * O)
        heads = list(range(g * HPC, (g + 1) * HPC))
        smalls = np.zeros((128, 2 + ST + HPC), np.float32)
        smalls[:, 0:2] = (bq[osl] * scale).reshape(2, 128).T
        smalls[:, 2 : 2 + ST] = maskbias[b].reshape(ST, 128).T
        smalls[0:64, 2 + ST :] = alpha[heads][None, :]
        def sb_layout(mat_T, kt):
            # [rows, cols] -> [128, kt, cols]: row r = k*128 + p -> [p][k]
            r, cols = mat_T.shape
            return np.ascontiguousarray(
                mat_T.reshape(kt, 128, cols).transpose(1, 0, 2)
            )

        in_maps.append(
            {
                "xT": sb_layout(x[b].T, KD),
                "wqT": sb_layout((Wq[osl] * scale).T, KD),
                "wkT": sb_layout(Wk[osl].T, KD),
                "wvT": sb_layout(Wv[osl].T, KD),
                "woT": sb_layout(Wo[:, osl].T, 2),
                "peT": peT_np,
                "smalls": smalls,
                "out": np.zeros((S, D), np.float32),
            }
        )

    _CACHE["in_maps"] = in_maps
    res = run_bass_kernel_spmd(nc, in_maps, core_ids=list(range(NCORES)))

    correction = Wo @ bv + bo  # exact bv/bo contribution (see module docstring)
    out = np.empty((B, S, D), np.float32)
    for b in range(B):
        acc = np.zeros((S, D), np.float64)
        for g in range(GROUPS):
            acc += res.results[b * GROUPS + g]["out"]
        out[b] = (acc + correction).astype(np.float32)
    return out



# revision 5
# speedup vs baseline: 730.6655x; 1.1682x over previous
"""DisentangledAttention on 8 Trainium2 cores (Bass/Tile).

Problem (hardcoded shapes): x[2,2048,1024], 16 heads x 64 dims, torch-Linear
projections, scores = q@k^T/8 + alpha_h*(pe@pe^T) + beta_h, key-side mask,
softmax, out = (P@v) @ Wo^T + bo.

Sharding: core i = (batch b = i//4, head-group g = i%4, heads 4g..4g+3).
Each core computes its 4 heads' attention and a partial out-projection
[2048,1024]; host sums the 4 partials per batch (tensor-parallel unshard).

Math simplifications (exact):
- beta_h is constant along the softmax axis -> cancels. Dropped.
- bk shifts scores by q.bk, constant along key axis -> cancels. Dropped.
- bv contributes sum_k P[q,k] * (bv @ Wo_slice^T) = bv @ Wo^T per row since
  softmax rows sum to 1 -> exact host-side additive correction with bo.
- bq enters scores via bq.k -> per-partition scalar add on q^T (DVE).
- 1/sqrt(64) folded into Wq on host; alpha_h applied on device (ACT scale).
- pos term fused into the QK matmul: q' = [q/8 ; alpha_h*pe], k' = [k ; pe]
  stacked along the contraction dim (64+64=128) -> pos attention is free.
- scores are built TRANSPOSED [key, query]: the key-side mask becomes a
  per-partition ACT bias on the exp, and P~^T feeds the PV matmul directly
  (no transpose). Softmax denominators come from a ones-row appended to V
  (M=65 PV matmul); normalization is a reciprocal + GPSIMD partition
  broadcast + multiply, entirely off the PE's critical path.
- no max-subtraction in softmax: scores ~ N(0,1) here, exp is f32-safe, and
  softmax is shift-invariant so this matches the reference to rounding.

Matmuls run in float32r (TF32-like, 11 mantissa bits, 4x faster than fp32
on the PE). PSUM accumulation is fp32. The out-projection is interleaved
into the (ACT-exp-paced) attention stream through a unified 4-slot PSUM
pool, making it nearly free on the PE timeline.
"""

import numpy as np

import concourse.bacc as bacc
import concourse.bass as bass
import concourse.mybir as mybir
import concourse.tile as tile
from concourse.bass import ds, ts
from concourse.bass_utils import run_bass_kernel_spmd

B = 2
S = 2048
D = 1024
H = 16
DH = 64
NCORES = 8
GROUPS = 4          # head-groups
HPC = H // GROUPS   # heads per core = 4
O = HPC * DH        # out dims per core = 256
KD = D // 128       # contraction tiles over d_model = 8
ST = S // 128       # seq tiles of 128 = 16
SC = S // 512       # seq chunks of 512 = 4

F32 = mybir.dt.float32
F32R = mybir.dt.float32r
BF16 = mybir.dt.bfloat16
BF16_NP = mybir.dt.np(BF16)

_CACHE = {}


class _null_ctx:
    def __enter__(self):
        return None

    def __exit__(self, *a):
        return False


def _build(reps: int = 1, loop_n: int | None = None):
    """Build the kernel program.

    reps: unrolled copies of the body (bench uses >1 to amortize).
    loop_n: if set, wrap the reps in a For_i hardware loop of this count
    (bench-only — lets timing loops run long enough to dominate the
    ~30-90ms tunnel dispatch jitter without exploding program size).
    """
    nc = bacc.Bacc("TRN2", target_bir_lowering=False, debug=False, num_devices=NCORES)

    # x / weights arrive pre-laid-out by the host in exactly the SBUF tile
    # shape ([partition, k-tile, free]), so each loads with ONE fully
    # contiguous DMA
    xT = nc.dram_tensor("xT", [128, KD, S], BF16, kind="ExternalInput").ap()
    wqT = nc.dram_tensor("wqT", [128, KD, O], BF16, kind="ExternalInput").ap()
    wkT = nc.dram_tensor("wkT", [128, KD, O], BF16, kind="ExternalInput").ap()
    wvT = nc.dram_tensor("wvT", [128, KD, O], BF16, kind="ExternalInput").ap()
    woT = nc.dram_tensor("woT", [128, 2, D], BF16, kind="ExternalInput").ap()
    peT = nc.dram_tensor("peT", [DH, S], BF16, kind="ExternalInput").ap()
    # smalls: [:, 0:2] = bq/8 by head-pair, [:, 2:18] = mask bias by key
    # tile, [0:64, 18:22] = per-head alpha replicated down 64 partitions
    smalls_d = nc.dram_tensor("smalls", [128, 2 + ST + HPC], F32, kind="ExternalInput").ap()
    out_d = nc.dram_tensor("out", [S, D], F32, kind="ExternalOutput").ap()

    with tile.TileContext(nc) as tc:
     with tc.For_i(0, loop_n, 1) if loop_n else _null_ctx() as _i:
      for _rep in range(reps):
        with tc.tile_pool(name="const", bufs=1) as const:
            woT_sb = const.tile([128, 2, D], BF16)
            smalls = const.tile([128, 2 + ST + HPC], F32)
            bqsb = smalls[:, 0:2]
            maskb = smalls[:, 2 : 2 + ST]
            alphas = smalls[0:64, 2 + ST : 2 + ST + HPC]
            peT_sb = const.tile([DH, S], BF16)

            # q'/k' per head ([128, S]: content half + pos half stacked along
            # the contraction dim), V' per head ([128 keys, 64+1] per key tile)
            with tc.tile_pool(name="qkv", bufs=1) as qkv:
                qp = [qkv.tile([128, S], BF16, name=f"qp{h}") for h in range(HPC)]
                kp = [qkv.tile([128, S], BF16, name=f"kp{h}") for h in range(HPC)]
                vp = qkv.tile([128, ST, HPC, DH + 1], BF16)
                nc.vector.memset(vp[:, :, :, DH : DH + 1], 1.0)

                # ---- projections ----
                with tc.tile_pool(name="proj", bufs=1) as proj:
                    xT_sb = proj.tile([128, KD, S], BF16)
                    wq_sb = proj.tile([128, KD, O], BF16)
                    wk_sb = proj.tile([128, KD, O], BF16)
                    wv_sb = proj.tile([128, KD, O], BF16)
                    # DMA order = need order: x/weights gate the first
                    # matmuls; pos-embed and Wo are consumed much later
                    nc.sync.dma_start(out=xT_sb, in_=xT)
                    for w_sb, w_d in ((wq_sb, wqT), (wk_sb, wkT), (wv_sb, wvT)):
                        nc.sync.dma_start(out=w_sb, in_=w_d)
                    nc.sync.dma_start(out=smalls, in_=smalls_d)
                    nc.sync.dma_start(out=peT_sb, in_=peT)
                    # pos halves: even head of a pair keeps content in rows
                    # 0:64 / pos in 64:128, odd head the reverse (both sides
                    # of the QK matmul use the same order, so dots match).
                    # q side is scaled by the head's alpha (data-driven)
                    for h in range(HPC):
                        crow = (h % 2) * 64          # content rows base
                        prow = 64 - crow             # pos rows base
                        nc.scalar.mul(
                            qp[h][prow : prow + 64, :],
                            peT_sb,
                            alphas[:, h : h + 1],
                        )
                        nc.scalar.copy(
                            out=kp[h][prow : prow + 64, :],
                            in_=peT_sb,
                        )
                    nc.sync.dma_start(out=woT_sb, in_=woT)

                    with tc.tile_pool(name="pps", bufs=4, space="PSUM") as pps:
                        # q^T and k^T by head-pair: psum [128 (2 heads), 512]
                        for hp in range(2):
                            for c in range(SC):
                                q_ps = pps.tile([128, 512], F32, tag="qk_ps")
                                for kd in range(KD):
                                    nc.tensor.matmul(
                                        out=q_ps,
                                        lhsT=wq_sb[:, kd, ts(hp, 128)],
                                        rhs=xT_sb[:, kd, ds(c * 512, 512)],
                                        start=(kd == 0),
                                        stop=(kd == KD - 1),
                                    )
                                for par in range(2):  # even/odd head of pair
                                    h = 2 * hp + par
                                    crow = (h % 2) * 64
                                    nc.vector.tensor_scalar_add(
                                        qp[h][crow : crow + 64, ds(c * 512, 512)],
                                        q_ps[crow : crow + 64, :],
                                        bqsb[crow : crow + 64, hp : hp + 1],
                                    )
                                k_ps = pps.tile([128, 512], F32, tag="qk_ps")
                                for kd in range(KD):
                                    nc.tensor.matmul(
                                        out=k_ps,
                                        lhsT=wk_sb[:, kd, ts(hp, 128)],
                                        rhs=xT_sb[:, kd, ds(c * 512, 512)],
                                        start=(kd == 0),
                                        stop=(kd == KD - 1),
                                    )
                                for par in range(2):
                                    h = 2 * hp + par
                                    crow = (h % 2) * 64
                                    nc.vector.tensor_copy(
                                        out=kp[h][crow : crow + 64, ds(c * 512, 512)],
                                        in_=k_ps[crow : crow + 64, :],
                                    )
                        # v: [seq, o] directly
                        for st in range(ST):
                            v_ps = pps.tile([128, O], F32, tag="v_ps")
                            for kd in range(KD):
                                nc.tensor.matmul(
                                    out=v_ps,
                                    lhsT=xT_sb[:, kd, ts(st, 128)],
                                    rhs=wv_sb[:, kd, :],
                                    start=(kd == 0),
                                    stop=(kd == KD - 1),
                                )
                            nc.vector.tensor_copy(
                                out=vp[:, st, :, 0:DH],
                                in_=v_ps.rearrange("p (h d) -> p h d", h=HPC),
                            )

                # ---- attention (scores transposed [key, query]) ----
                # sq handled in chunks of 1024 (CW) so exp runs as [128, 1024]
                # ACT instructions, halving the per-instruction access bubble
                CW = 1024
                NCH = S // CW
                attnT = [qkv.tile([128, S], BF16, name=f"attnT{kt}") for kt in range(2)]
                # single unified PSUM pool: scores, z-accumulators, and
                # out-proj tiles are all 2 banks, so one 4-slot pool (8 banks)
                # lets outproj cycle through the spare slot without starving
                # the score ping-pong that paces ACT
                with (
                    tc.tile_pool(name="att", bufs=3) as att,
                    tc.tile_pool(name="nrm", bufs=2) as nrm,
                    tc.tile_pool(name="ups", bufs=4, space="PSUM") as ups,
                ):
                    def emit_outproj(st_range):
                        # partial out-projection (host sums over head-groups).
                        # Emitted per sq half as soon as all heads' attnT
                        # columns are done: this PE work runs inside the
                        # ACT(exp)-paced attention stream, so it's ~free.
                        # o_ps shares the score pool slots (free between
                        # chunks) to stay within the 8 PSUM banks. Two
                        # s-tiles share one staging tile and one DMA.
                        sts = list(st_range)
                        for st0 in sts[::2]:
                            o_sb2 = att.tile([128, 2, D], F32, tag="osb2")
                            for j in range(2):
                                st = st0 + j
                                o_ps = ups.tile([128, D], F32, tag="u", name="o_ps")
                                for nk in range(2):
                                    for kt in range(2):
                                        nc.tensor.matmul(
                                            out=o_ps[:, ds(nk * 512, 512)],
                                            lhsT=attnT[kt][:, ts(st, 128)],
                                            rhs=woT_sb[:, kt, ds(nk * 512, 512)],
                                            start=(kt == 0),
                                            stop=(kt == 1),
                                        )
                                nc.vector.tensor_copy(out=o_sb2[:, j, :], in_=o_ps)
                            nc.sync.dma_start(
                                out=out_d[ds(st0 * 128, 256), :].rearrange(
                                    "(two p) d -> p two d", p=128
                                ),
                                in_=o_sb2,
                            )

                    for c in range(NCH):
                        for h in range(HPC):
                            if c > 0 and h > 0:
                                # previous chunk's outproj, spread in small
                                # waves one head into this chunk: the
                                # dependency (previous chunk's last
                                # normalize) is long done, and small waves
                                # limit score-slot contention
                                w0, w1 = [(0, 0), (0, 2), (2, 6), (6, 8)][h]
                                emit_outproj(range((c - 1) * 8 + w0, (c - 1) * 8 + w1))
                            z_ps = ups.tile([DH + 1, CW], F32, tag="u", name="z_ps")
                            for t in range(ST):
                                s_ps = ups.tile([128, CW], F32, tag="u", name="s_ps")
                                for half in range(CW // 512):
                                    nc.tensor.matmul(
                                        out=s_ps[:, ds(half * 512, 512)],
                                        lhsT=kp[h][:, ts(t, 128)],
                                        rhs=qp[h][:, ds(c * CW + half * 512, 512)],
                                        start=True,
                                        stop=True,
                                    )
                                p_sb = att.tile([128, CW], BF16, tag="p")
                                nc.scalar.activation(
                                    out=p_sb,
                                    in_=s_ps,
                                    func=mybir.ActivationFunctionType.Exp,
                                    bias=maskb[:, t : t + 1],
                                    scale=1.0,
                                )
                                for half in range(CW // 512):
                                    nc.tensor.matmul(
                                        out=z_ps[:, ds(half * 512, 512)],
                                        lhsT=vp[:, t, h, :],
                                        rhs=p_sb[:, ds(half * 512, 512)],
                                        start=(t == 0),
                                        stop=(t == ST - 1),
                                    )
                            recip = nrm.tile([1, CW], F32, tag="recip")
                            nc.vector.reciprocal(recip, z_ps[DH : DH + 1, :])
                            # broadcast 1/denom to 64 partitions on GPSIMD
                            # (keeps the whole normalize chain off the PE)
                            bc_sb = nrm.tile([64, CW], F32, tag="bc_sb")
                            nc.gpsimd.partition_broadcast(bc_sb, recip)
                            row = (h % 2) * 64
                            nc.vector.tensor_mul(
                                out=attnT[h // 2][row : row + 64, ds(c * CW, CW)],
                                in0=z_ps[0:DH, :],
                                in1=bc_sb,
                            )
                    emit_outproj(range((NCH - 1) * 8, NCH * 8))

    nc.compile()
    return nc


def kernel(
    x, mask, Wq, bq, Wk, bk, Wv, bv, Wo, bo, pos_embed, alpha, beta, **_unused
):
    x = np.asarray(x, dtype=np.float32)
    mask = np.asarray(mask)
    Wq = np.asarray(Wq, dtype=np.float32)
    Wk = np.asarray(Wk, dtype=np.float32)
    Wv = np.asarray(Wv, dtype=np.float32)
    Wo = np.asarray(Wo, dtype=np.float32)
    bq = np.asarray(bq, dtype=np.float32)
    bv = np.asarray(bv, dtype=np.float32)
    bo = np.asarray(bo, dtype=np.float32)
    pe = np.asarray(pos_embed, dtype=np.float32)
    alpha = np.asarray(alpha, dtype=np.float32).reshape(H)

    if "nc" not in _CACHE:
        _CACHE["nc"] = _build()
    nc = _CACHE["nc"]

    scale = np.float32(1.0 / np.sqrt(DH))
    peT_np = np.ascontiguousarray(pe.T)
    maskbias = np.where(mask == 0, np.float32(-1e9), np.float32(0.0)).astype(np.float32)

    in_maps = []
    for core in range(NCORES):
        b, g = divmod(core, GROUPS)
        osl = slice(g * O, (g + 1) * O)
        heads = list(range(g * HPC, (g + 1) * HPC))
        smalls = np.zeros((128, 2 + ST + HPC), np.float32)
        smalls[:, 0:2] = (bq[osl] * scale).reshape(2, 128).T
        smalls[:, 2 : 2 + ST] = maskbias[b].reshape(ST, 128).T
        smalls[0:64, 2 + ST :] = alpha[heads][None, :]
        def sb_layout(mat_T, kt):
            # [rows, cols] -> [128, kt, cols]: row r = k*128 + p -> [p][k]
            r, cols = mat_T.shape
            return np.ascontiguousarray(
                mat_T.reshape(kt, 128, cols).transpose(1, 0, 2)
            )

        in_maps.append(
            {
                "xT": sb_layout(x[b].T, KD).astype(BF16_NP),
                "wqT": sb_layout((Wq[osl] * scale).T, KD).astype(BF16_NP),
                "wkT": sb_layout(Wk[osl].T, KD).astype(BF16_NP),
                "wvT": sb_layout(Wv[osl].T, KD).astype(BF16_NP),
                "woT": sb_layout(Wo[:, osl].T, 2).astype(BF16_NP),
                "peT": peT_np.astype(BF16_NP),
                "smalls": smalls,
                "out": np.zeros((S, D), np.float32),
            }
        )

    _CACHE["in_maps"] = in_maps
    res = run_bass_kernel_spmd(nc, in_maps, core_ids=list(range(NCORES)))

    correction = Wo @ bv + bo  # exact bv/bo contribution (see module docstring)
    out = np.empty((B, S, D), np.float32)
    for b in range(B):
        acc = np.zeros((S, D), np.float64)
        for g in range(GROUPS):
            acc += res.results[b * GROUPS + g]["out"]
        out[b] = (acc + correction).astype(np.float32)
    return out

